# revision 32
# baseline (speedup 1.0000x reference)
"""Trainium2 Bass kernel for nn_CLEAR_45561013076524 (vq_codebook).

Pure data-parallel over 8 NeuronCores: each core computes 512 images of the
conv-encoder -> SoftSOM -> (collapsed) classifier pipeline.

Mathematical simplifications (validated numerically against the reference):
  * The node-attention block has n_nodes=1, so its softmax is identically 1
    and `fused == blended` tiled 4x.  Therefore
       logits = blended @ sum_h clf_w[h*256:(h+1)*256] + clf_b
    and y/class_emb/query_*/attn_*/node_emb are dead inputs.
  * conv1 (5x5, stride 2, pad 1) is one K=75 im2col matmul (host-built
    patches), zero-padded to K=128 so the PE stays at its full 2.4 GHz
    p-state (K<128 throttles the array to 1.2 GHz, measured).
  * cdist^2 is computed as one accumulated matmul chain by augmenting the
    contraction with ones/|z|^2 rows against |c|^2/ones columns and
    pre-scaling c^T by -2.

Matmul convention: out[M,N] = lhsT[K,M].T @ rhs[K,N], K on SBUF partitions.

Perf structure (all measured on HW via microbenchmarks):
  * The PE throttles to 1.2 GHz whenever fewer than ~128 array rows are
    active, so every matmul in the hot path is arranged to keep all 128
    rows busy (zero-padding K and zero-masked weight rows where needed).
  * conv1 lhsT is [128,128] = 4 column-stacked copies of the weights, so the
    single matmul also materializes 4 replicas of h1 across the 4 partition
    quadrants -- free input replication for conv2's row-tiling.
  * conv2/conv3: the 9 taps run on 2 concurrent 64-row PE tiles via
    tile_position=(64i,0) (conv2's K=32 zero-padded to 64), accumulating
    into 2 PSUM banks that are merged at eviction (ACT copy + DVE add +
    DVE fused bias-relu, chosen to balance the two PSUM-capable engines).
  * conv4 is K=128/M=128/N=512 at full-clock cadence (~248ns incl. the
    3-dim access-pattern walker overhead).
  * enc runs "swapped": h4 column tiles are the stationary operand and
    enc_w streams as the moving operand with N=256, so the per-matmul
    weight load hides under the matmul; the phase is enc_w-DMA-bound.
  * The SoftSOM tail is split into distance / softmax / transpose passes so
    the in-order PE never stalls behind the ACT/DVE softmax chain.
"""

import numpy as np
import ml_dtypes

import concourse.bass as bass
from concourse import bacc
from concourse import mybir
from concourse.tile import TileContext
from concourse.bass_utils import run_bass_kernel_spmd
from concourse.masks import make_identity

BF16NP = ml_dtypes.bfloat16
F8NP = ml_dtypes.float8_e4m3
DRPM = mybir.MatmulPerfMode.DoubleRow
W3 = 12                   # padded x width of the h3 [c, y, x, img] layout
F32 = mybir.dt.float32
F32R = mybir.dt.float32r
BF = mybir.dt.bfloat16
F8 = mybir.dt.float8e4
AF = mybir.ActivationFunctionType
ALU = mybir.AluOpType

NCORES = 8
B = 4096
BL = B // NCORES          # images per core
CH = 64                   # chunk (images) for conv3/conv4
SUB = 32                  # sub-chunk for conv1/conv2
PAIR = 2 * CH             # images per enc pass

OFF9 = [(ky, kx) for ky in range(3) for kx in range(3)]

# DR tap-coverage tables: group shifts (sigma) and per-matmul (base, delta)
W1X = 18                  # x pitch of h1 [c, 17y, 18x, img]
SIG2 = [(0, 0), (0, 1), (1, 0), (1, 1)]
M2 = [((0, 0), (2, 0)), ((0, 2), (2, 0))]
SIG3 = [(0, 0), (1, 0)]
M3 = [((0, 0), (0, 1)), ((0, 2), (2, -2)), ((2, 1), (0, 1))]

_CACHE = {}


# --------------------------------------------------------------------------
# host-side input preparation (layout only / tiny parameter math)
# --------------------------------------------------------------------------

def _prep_host(inputs):
    f32 = np.float32
    x = np.ascontiguousarray(np.asarray(inputs['x'], f32))
    xp = np.zeros((B, 3, 34, 34), f32)
    xp[:, :, 1:33, 1:33] = x
    # conv1 im2col on host (pure gather): xim[b, (ci,ky,kx), (oy,ox)]
    from numpy.lib.stride_tricks import sliding_window_view
    win = sliding_window_view(xp, (5, 5), axis=(2, 3))[:, :, ::2, ::2]
    xim = np.zeros((B, 128, 225), F8NP)     # K pre-padded to 128 rows
    xim[:, :75] = win.transpose(0, 1, 4, 5, 2, 3).reshape(
        B, 75, 225).astype(F8NP)

    c1w = np.asarray(inputs['conv1_w'], f32)
    w1 = c1w.transpose(1, 2, 3, 0).reshape(75, 32)
    w1p = np.zeros((128, 128), f32)           # K padded to 128, M tiled 4x
    for g in range(4):
        w1p[:75, 32 * g:32 * g + 32] = w1
    w1p = w1p.astype(F8NP)
    b1r = np.tile(np.asarray(inputs['conv1_b'], f32), 4).reshape(128, 1)

    # conv2/conv3 as fp8 DoubleRow matmuls over the [c, y, x, img] layout.
    # Partition group g of the input holds a copy of the activation shifted
    # by sigma_g; a DR matmul at base tap (by,bx) with slot delta (dy,dx)
    # covers taps (by+dy*s+sy_g, bx+dx*s+sx_g) -- weights of out-of-range
    # taps are zeroed.  Coverage of all 9 taps is asserted below.
    def pack_dr(w, SIG, MM, gsz):
        nmm = len(MM)
        out = np.zeros((nmm, 128, 2, w.shape[0]), f32)
        used = []
        for m, ((by, bx), (dy, dx)) in enumerate(MM):
            for s in range(2):
                for g, (sy, sx) in enumerate(SIG):
                    ky, kx = by + dy * s + sy, bx + dx * s + sx
                    if 0 <= ky < 3 and 0 <= kx < 3:
                        out[m, gsz * g:gsz * (g + 1), s] = w[:, :, ky, kx].T
                        used.append((ky, kx))
        assert sorted(used) == [(a, b) for a in range(3) for b in range(3)]
        # partition dim first: [128, nmm, 2, M]
        return np.ascontiguousarray(out.transpose(1, 0, 2, 3)).astype(F8NP)

    # conv2 weights M-stacked 2x: PSUM rows 64-127 are a copy that gets
    # evicted straight into h2's shifted-replica partition group.
    w2n1 = pack_dr(np.asarray(inputs['conv2_w'], f32), SIG2, M2, 32)
    w2n = np.concatenate([w2n1, w2n1], axis=3)             # [128, 2, 2, 128]
    b2r = np.tile(np.asarray(inputs['conv2_b'], f32), 2).reshape(128, 1)
    w3n = pack_dr(np.asarray(inputs['conv3_w'], f32), SIG3, M3, 64)
    b3r = np.asarray(inputs['conv3_b'], f32).reshape(128, 1)

    w4f = np.asarray(inputs['conv4_w'], f32)                # [256,128,3,3]
    w4 = np.ascontiguousarray(
        w4f.reshape(2, 128, 128, 3, 3).transpose(2, 3, 4, 0, 1)
        .reshape(128, 9, 2, 128)).astype(F8NP)
    b4 = np.ascontiguousarray(
        np.asarray(inputs['conv4_b'], f32).reshape(2, 128).T)  # [128,2]

    # enc (swapped): moving operand encw_m[ct, y, c, x, dout]
    ew = np.asarray(inputs['enc_w'], f32).reshape(2, 128, 8, 8, 256)
    encw = np.ascontiguousarray(ew.transpose(0, 2, 1, 3, 4)).astype(F8NP)
    encb = np.broadcast_to(np.asarray(inputs['enc_b'], f32), (128, 256)).copy()

    nf = np.asarray(inputs['node_fc_w'], f32).reshape(2, 128, 2, 128)
    nfc = np.ascontiguousarray(nf.transpose(1, 0, 2, 3))       # [k,kt,mt,m]
    nfcb = np.ascontiguousarray(
        np.asarray(inputs['node_fc_b'], f32).reshape(2, 128).T)

    protos = np.asarray(inputs['protos'], f32)
    grid = np.asarray(inputs['grid_pos'], f32)

    def dist_rhs(c):
        # rp[k, kt, n] = -2*c[n, kt*128+k]
        # aug (K padded to 128 to keep the PE p-state up):
        #   row0 = |c|^2 (pairs with the all-ones lhsT row)
        #   row1 = ones  (pairs with the |z|^2 lhsT row)
        rp = np.ascontiguousarray(
            (-2.0 * c.T).reshape(2, 128, 256).transpose(1, 0, 2))
        aug = np.zeros((128, 256), f32)
        aug[0] = (c * c).sum(1)
        aug[1] = 1.0
        return rp.astype(f32), aug.astype(f32)

    rp, rpa = dist_rhs(protos)
    rg, rga = dist_rhs(grid)

    # blended is only consumed by the classifier, so fold protos into it:
    # logits = w_norm @ (protos @ clf_sum) + clf_b
    clf_sum = np.asarray(inputs['clf_w'], f32).reshape(4, 256, 10).sum(0)
    pc = (protos.astype(np.float64) @ clf_sum.astype(np.float64)).astype(f32)
    clfs = np.ascontiguousarray(
        pc.reshape(2, 128, 10).transpose(1, 0, 2))             # [128, 2, 10]
    clfb = np.broadcast_to(np.asarray(inputs['clf_b'], f32), (128, 10)).copy()

    gate = 1.0 / (1.0 + np.exp(-np.asarray(inputs['gate_logits'], np.float64)))
    gateb = np.broadcast_to(gate.astype(f32), (128, 256)).copy()

    traw = float(np.asarray(inputs['temp_raw']).reshape(-1)[0])
    temp = 1.0 / (1.0 + np.exp(-traw)) * (1.0 - 0.001) + 0.001
    invt = np.full((128, 1), 1.0 / temp, f32)
    ninvt = np.full((128, 1), -1.0 / temp, f32)

    shared = dict(w1=w1p, w2=w2n, w3=w3n, w4=w4,
                  onesr=np.ones((1, 512), f32), onescol=np.ones((128, 1), f32),
                  b1=b1r, b2=b2r, b3=b3r,
                  b4=b4, encw=encw, encb=encb, nfc=nfc, nfcb=nfcb,
                  rp=rp, rpa=rpa, rg=rg, rga=rga,
                  clfs=clfs, clfb=clfb, gateb=gateb, invt=invt, ninvt=ninvt)
    return xim, shared


# --------------------------------------------------------------------------
# device program
# --------------------------------------------------------------------------

def _build_nc():
    nc = bacc.Bacc(None, target_bir_lowering=False)
    P = nc.declare_dram_parameter
    xim = P("xim", [BL, 128, 225], F8, isOutput=False)
    w1 = P("w1", [128, 128], F8, isOutput=False)
    w2 = P("w2", [128, 2, 2, 128], F8, isOutput=False)
    w3 = P("w3", [128, 3, 2, 128], F8, isOutput=False)
    w4 = P("w4", [128, 9, 2, 128], F8, isOutput=False)
    b1 = P("b1", [128, 1], F32, isOutput=False)
    b2 = P("b2", [128, 1], F32, isOutput=False)
    b3 = P("b3", [128, 1], F32, isOutput=False)
    b4 = P("b4", [128, 2], F32, isOutput=False)
    encw = P("encw", [2, 8, 128, 8, 256], F8, isOutput=False)
    encb = P("encb", [128, 256], F32, isOutput=False)
    nfc = P("nfc", [128, 2, 2, 128], F32, isOutput=False)
    nfcb = P("nfcb", [128, 2], F32, isOutput=False)
    rp = P("rp", [128, 2, 256], F32, isOutput=False)
    rpa = P("rpa", [128, 256], F32, isOutput=False)
    rg = P("rg", [128, 2, 256], F32, isOutput=False)
    rga = P("rga", [128, 256], F32, isOutput=False)
    clfs = P("clfs", [128, 2, 10], F32, isOutput=False)
    clfb = P("clfb", [128, 10], F32, isOutput=False)
    gateb = P("gateb", [128, 256], F32, isOutput=False)
    invt = P("invt", [128, 1], F32, isOutput=False)
    onesr = P("onesr", [1, 512], F32, isOutput=False)
    onescol = P("onescol", [128, 1], F32, isOutput=False)
    ninvt = P("ninvt", [128, 1], F32, isOutput=False)
    outd = P("out", [BL, 10], F32, isOutput=True)

    with TileContext(nc) as tc:
        with (tc.tile_pool(name="consts", bufs=1) as consts,
              tc.tile_pool(name="acts", bufs=1) as acts,
              tc.tile_pool(name="encwp", bufs=6) as encwp,
              tc.tile_pool(name="cvtmp", bufs=6) as cvtmp,
              tc.tile_pool(name="smp", bufs=3) as smp,
              tc.tile_pool(name="stats", bufs=8) as stats,
              tc.tile_pool(name="outp", bufs=2) as outp,
              tc.tile_pool(name="psA", bufs=6, space="PSUM") as psA,
              tc.tile_pool(name="psB", bufs=2, space="PSUM") as psB):

            dma = nc.sync.dma_start

            # ---- conv1-critical loads first (everything else overlaps) ----
            w1s = consts.tile([128, 128], F8); dma(out=w1s, in_=w1[:])
            b1s = consts.tile([128, 1], F32); dma(out=b1s, in_=b1[:])
            pts = []
            for i in range(2):
                t = acts.tile([128, SUB, 15, 15], F8, name=f"pt{i}")
                pts.append(t)

            def load_patches(b0, pt):
                base = xim[b0, 0, 0]
                src = bass.AP(
                    tensor=base.tensor, offset=base.offset,
                    ap=[[225, 128], [128 * 225, SUB], [1, 225]])
                dma(out=pt[:], in_=src)

            load_patches(0, pts[0])

            # ---- remaining constants --------------------------------------
            w2s = consts.tile([128, 2, 2, 128], F8); dma(out=w2s, in_=w2[:])
            w3s = consts.tile([128, 3, 2, 128], F8); dma(out=w3s, in_=w3[:])
            w4s = consts.tile([128, 9, 2, 128], F8); dma(out=w4s, in_=w4[:])
            b2s = consts.tile([128, 1], F32); dma(out=b2s, in_=b2[:])
            b3s = consts.tile([128, 1], F32); dma(out=b3s, in_=b3[:])
            b4s = consts.tile([128, 2], F32); dma(out=b4s, in_=b4[:])
            encbs = consts.tile([128, 256], F32); dma(out=encbs, in_=encb[:])
            ident = consts.tile([128, 128], F32)
            make_identity(nc, ident)

            # ---- persistent activation tensors ----------------------------
            # h1 is [c, y, img, x] (x-innermost so conv1 evictions write 15B
            # runs); h2/h3 are [c, y, x, img].  Partition group g holds the
            # activation shifted by sigma_g, filled by flat-shift SBUF-SBUF
            # DMAs (pad columns are zero, so the img-boundary wrap of the
            # flat shift lands on values that are only read via zero
            # weights).
            h1na = []
            for i in range(2):
                t = acts.tile([128, 17, CH, W1X], F8, name=f"h1n{i}")
                nc.vector.memset(t, 0.0)
                h1na.append(t)
            h2n = acts.tile([128, 10, W3, CH], F8)
            nc.gpsimd.memset(h2n, 0.0)
            h3n = acts.tile([128, 10, W3, CH], F8)     # [c, y, x, img]
            nc.gpsimd.memset(h3n, 0.0)
            h4t = acts.tile([128, 2, 8, 8, PAIR], F8)  # [c, ct, y, x, b]
            z0b = acts.tile([128, 4, 256], F32)        # [b, pair, dout]
            z0T = acts.tile([128, 2, BL], F32)
            zT = acts.tile([128, 2, BL], F32)
            wT = acts.tile([128, 2, BL], F32)

            # softsom constants on the gpsimd queue so they overlap the conv
            # pipeline without delaying patch loads on the sync queue
            gdma = nc.gpsimd.dma_start
            nfcs = consts.tile([128, 2, 2, 128], F32); gdma(out=nfcs, in_=nfc[:])
            nfcbs = consts.tile([128, 2], F32); gdma(out=nfcbs, in_=nfcb[:])
            rps = consts.tile([128, 2, 256], F32); gdma(out=rps, in_=rp[:])
            rpas = consts.tile([128, 256], F32); gdma(out=rpas, in_=rpa[:])
            rgs = consts.tile([128, 2, 256], F32); gdma(out=rgs, in_=rg[:])
            rgas = consts.tile([128, 256], F32); gdma(out=rgas, in_=rga[:])
            clfss = consts.tile([128, 2, 10], F32); gdma(out=clfss, in_=clfs[:])
            clfbs = consts.tile([128, 10], F32); gdma(out=clfbs, in_=clfb[:])
            gatebs = consts.tile([128, 256], F32); gdma(out=gatebs, in_=gateb[:])
            invts = consts.tile([128, 1], F32); gdma(out=invts, in_=invt[:])
            ninvts = consts.tile([128, 1], F32); gdma(out=ninvts, in_=ninvt[:])
            ones_col = consts.tile([128, 1], F32)
            gdma(out=ones_col, in_=onescol[:])
            z2row = consts.tile([1, BL], F32)    # |z|^2 per image
            aug2 = consts.tile([128, BL], F32)   # K-padded aug lhsT
            nc.vector.memset(aug2, 0.0)
            gdma(out=aug2[0:1], in_=onesr[:])

            rowsz = CH * W1X
            wns = {}

            def tail_b(p):
                # transposes + logits + out for a tile whose softmax chain
                # finished during the preceding chunks (PE never waits)
                bs = slice(p * 128, (p + 1) * 128)
                wn = wns.pop(p)
                for kt in range(2):
                    tp = psA.tile([128, 128], F32, tag="ps")
                    nc.tensor.transpose(
                        tp[:], wn[:, kt * 128:(kt + 1) * 128], ident[:])
                    nc.vector.tensor_copy(out=wT[:, kt, bs], in_=tp[:])
                lg = psA.tile([128, 10], F32, tag="ps")
                for kt in range(2):
                    nc.tensor.matmul(lg[:], wT[:, kt, bs], clfss[:, kt],
                                     start=(kt == 0), stop=(kt == 1))
                ot = outp.tile([128, 10], F32)
                nc.vector.tensor_add(ot, lg[:], clfbs)
                dma(out=outd[p * 128:(p + 1) * 128], in_=ot)

            def conv1_phase(c):
                # conv1: single K=128(padded) matmul per image pair, evicted
                # into partition group 0 of this chunk's h1 buffer; groups
                # 1..3 (sigma-shifted replicas) filled by flat-shift DMAs on
                # the scalar HWDGE queue, issued per s-block so the first
                # half flies while the second half's matmuls run.
                hb = h1na[c % 2]
                for s in range(CH // SUB):
                    b0 = c * CH + s * SUB
                    pt = pts[(2 * c + s) % 2]
                    if c == 0 and s == 0:
                        for q in range(4):   # split so the first MM starts asap
                            base = xim[b0 + 8 * q, 0, 0]
                            src = bass.AP(
                                tensor=base.tensor, offset=base.offset,
                                ap=[[225, 128], [128 * 225, 8], [1, 225]])
                            dma(out=pt[:, 8 * q:8 * (q + 1)], in_=src)
                    else:
                        load_patches(b0, pt)
                    for j in range(SUB // 2):
                        ps1 = psA.tile([128, 2, 15, 15], F32, tag="ps")
                        nc.tensor.matmul(ps1[:], w1s[:], pt[:, 2 * j:2 * j + 2],
                                         start=True, stop=True)
                        ib = s * SUB + 2 * j
                        dst1 = hb[0:32, 1:16, ib:ib + 2, 1:16].rearrange(
                            "p y b x -> p b y x")
                        if j % 2 == 0:
                            nc.scalar.activation(out=dst1, in_=ps1[0:32],
                                                 func=AF.Relu,
                                                 bias=b1s[0:32, 0:1])
                        else:
                            nc.vector.tensor_scalar(
                                out=dst1, in0=ps1[0:32], scalar1=b1s[0:32, 0:1],
                                scalar2=0.0, op0=ALU.add, op1=ALU.max)
                    # this s-block's slice of the shifted replicas
                    seg = SUB * W1X
                    src0 = hb[0:32, 0, 0, 0]
                    for g, (sy, sx) in enumerate(SIG2[1:], start=1):
                        rows = 17 - sy
                        cnt = seg - sx
                        dstg = hb[32 * g:32 * (g + 1), 0, 0, 0]
                        nc.scalar.dma_start(
                            out=bass.AP(
                                tensor=dstg.tensor,
                                offset=dstg.offset + s * seg,
                                ap=[list(dstg.ap[0]), [rowsz, rows], [1, cnt]]),
                            in_=bass.AP(
                                tensor=src0.tensor,
                                offset=src0.offset + sy * rowsz + s * seg + sx,
                                ap=[list(src0.ap[0]), [rowsz, rows], [1, cnt]]))

            # ---- conv/enc pipeline over image chunks, conv1 one chunk
            # ahead so its replica DMAs hide under conv2..conv4 PE work ----
            conv1_phase(0)
            for c in range(BL // CH):
                pb = (c % 2) * CH
                h1c = h1na[c % 2]
                if c + 1 < BL // CH:
                    conv1_phase(c + 1)

                # conv2: stride-2 fp8 DoubleRow, 2 matmuls per output row;
                # N enumerates (img, x), the evictions transpose to (x, img).
                # M is double-stacked: PSUM rows 64-127 evict straight into
                # h2's shifted-replica partition group (one row up).
                for y in range(8):
                    p2 = psA.tile([128, CH, 8], F32, tag="ps", name="p2")
                    for m, ((by, bx), (dy, dx)) in enumerate(M2):
                        base = h1c[:, 2 * y + by, 0, bx]
                        delta = (dy * rowsz + dx)
                        rhs = bass.AP(
                            tensor=base.tensor, offset=base.offset,
                            ap=[list(base.ap[0]), [delta, 2],
                                [W1X, CH], [2, 8]])
                        nc.tensor.matmul(p2[:], w2s[:, m], rhs,
                                         start=(m == 0), stop=(m == len(M2) - 1),
                                         perf_mode=DRPM)
                    dst2 = h2n[0:64, 1 + y, 1:9, :]
                    src2 = p2[0:64].rearrange("p i x -> p x i")
                    dst2b = h2n[64:128, y, 1:9, :]
                    src2b = p2[64:128].rearrange("p i x -> p x i")
                    if y % 2 == 0:
                        nc.scalar.activation(out=dst2, in_=src2,
                                             func=AF.Relu, bias=b2s[0:64, 0:1])
                        nc.vector.tensor_scalar(
                            out=dst2b, in0=src2b, scalar1=b2s[64:128, 0:1],
                            scalar2=0.0, op0=ALU.add, op1=ALU.max)
                    else:
                        nc.vector.tensor_scalar(
                            out=dst2, in0=src2, scalar1=b2s[0:64, 0:1],
                            scalar2=0.0, op0=ALU.add, op1=ALU.max)
                        nc.scalar.activation(out=dst2b, in_=src2b,
                                             func=AF.Relu, bias=b2s[64:128, 0:1])

                # conv3: fp8 DoubleRow, 3 matmuls per output row
                for y in range(8):
                    p3 = psA.tile([128, 8, CH], F32, tag="ps", name="p3")
                    for m, ((by, bx), (dy, dx)) in enumerate(M3):
                        base = h2n[:, y + by, bx, 0]
                        delta = (dy * W3 + dx) * CH
                        rhs = bass.AP(
                            tensor=base.tensor, offset=base.offset,
                            ap=[list(base.ap[0]), [delta, 2],
                                [CH, 8], [1, CH]])
                        nc.tensor.matmul(p3[:], w3s[:, m], rhs,
                                         start=(m == 0), stop=(m == len(M3) - 1),
                                         perf_mode=DRPM)
                    dst3 = h3n[:, 1 + y, 1:9, :]
                    if y % 2 == 0:
                        nc.vector.tensor_scalar(
                            out=dst3, in0=p3[:], scalar1=b3s[:, 0:1],
                            scalar2=0.0, op0=ALU.add, op1=ALU.max)
                    else:
                        nc.scalar.activation(out=dst3, in_=p3[:],
                                             func=AF.Relu, bias=b3s[:, 0:1])

                # conv4: fp8 DoubleRow over tap pairs.  Per output row y,
                # 4 DR matmuls (flat taps (2i,2i+1), slot stride = the tap
                # offset delta in the [c,y,x,img] layout) + 1 plain fp8
                # matmul (tap 8) accumulate K=9*128 into one [128,8x,64b]
                # PSUM bank.
                for mt in range(2):
                    for y in range(8):
                        pa = psA.tile([128, 8, CH], F32, tag="ps",
                                      name="pc4")
                        for i in range(4):
                            ky0, kx0 = divmod(2 * i, 3)
                            ky1, kx1 = divmod(2 * i + 1, 3)
                            base = h3n[:, y + ky0, kx0, 0]
                            delta = ((ky1 - ky0) * W3 + (kx1 - kx0)) * CH
                            rhs = bass.AP(
                                tensor=base.tensor, offset=base.offset,
                                ap=[list(base.ap[0]), [delta, 2],
                                    [CH, 8], [1, CH]])
                            nc.tensor.matmul(pa[:],
                                             w4s[:, 2 * i:2 * i + 2, mt],
                                             rhs, start=(i == 0), stop=False,
                                             perf_mode=DRPM)
                        nc.tensor.matmul(pa[:], w4s[:, 8, mt],
                                         h3n[:, y + 2, 2:10, :],
                                         start=False, stop=True)
                        dst = h4t[:, mt, y, :, pb:pb + CH]
                        if y % 2 == 0:
                            nc.scalar.activation(
                                out=dst, in_=pa[:],
                                func=AF.Relu, bias=b4s[:, mt:mt + 1])
                        else:
                            nc.vector.tensor_scalar(
                                out=dst, in0=pa[:],
                                scalar1=b4s[:, mt:mt + 1], scalar2=0.0,
                                op0=ALU.add, op1=ALU.max)

                # enc (swapped): h4 column-tiles stationary, enc_w moving
                if c % 2 == 1:
                    p = c // 2
                    if p >= 1:
                        tail_b(p - 1)
                    zp = psB.tile([128, 256], F32, tag="pe")
                    for ct in range(2):
                        for y in range(8):
                            ewt = encwp.tile([128, 8, 256], F8)
                            nc.gpsimd.dma_start(out=ewt, in_=encw[ct, y])
                            for xx in range(8):
                                first = (ct == 0 and y == 0 and xx == 0)
                                last = (ct == 1 and y == 7 and xx == 7)
                                nc.tensor.matmul(
                                    zp[:], h4t[:, ct, y, xx], ewt[:, xx],
                                    start=first, stop=last)
                    nc.vector.tensor_add(z0b[:, p], zp[:], encbs)
                    # transpose this pair's z0 into z0T right away so it
                    # overlaps with the next chunks' conv work
                    for kt in range(2):
                        tp = psA.tile([128, 128], F32, tag="ps")
                        nc.tensor.transpose(
                            tp[:], z0b[:, p, 128 * kt:128 * kt + 128],
                            ident[:])
                        nc.vector.tensor_copy(
                            out=z0T[:, kt, p * 128:(p + 1) * 128], in_=tp[:])

                    # ---- SoftSOM tail for this 128-image tile, inline so
                    # it overlaps the remaining chunks' conv work ----------
                    bt = p
                    bs = slice(bt * 128, (bt + 1) * 128)
                    for mt in range(2):
                        zpm = psA.tile([128, 128], F32, tag="ps",
                                       name="zpm")
                        for kt in range(2):
                            nc.tensor.matmul(zpm[:], nfcs[:, kt, mt],
                                             z0T[:, kt, bs],
                                             start=(kt == 0), stop=(kt == 1))
                        nc.vector.tensor_scalar(out=zT[:, mt, bs], in0=zpm[:],
                                                scalar1=nfcbs[:, mt:mt + 1],
                                                scalar2=None, op0=ALU.add)
                    zp2 = psA.tile([1, 128], F32, tag="ps", name="zp2")
                    for kt in range(2):
                        sqk = cvtmp.tile([128, 128], F32, tag='sqk', bufs=2)
                        nc.scalar.activation(out=sqk, in_=zT[:, kt, bs],
                                             func=AF.Square)
                        nc.tensor.matmul(zp2[:], ones_col[:], sqk,
                                         start=(kt == 0), stop=(kt == 1))
                    nc.vector.tensor_copy(out=z2row[0:1, bs], in_=zp2[:])
                    dma(out=aug2[1:2, bs], in_=z2row[0:1, bs])

                    # distances
                    parts = []
                    for rmain, raug in ((rps, rpas), (rgs, rgas)):
                        dp = psA.tile([128, 256], F32, tag="ps",
                                      name=f"dp{bt}")
                        nc.tensor.matmul(dp[:], zT[:, 0, bs], rmain[:, 0],
                                         start=True, stop=False)
                        nc.tensor.matmul(dp[:], zT[:, 1, bs], rmain[:, 1],
                                         start=False, stop=False)
                        nc.tensor.matmul(dp[:], aug2[:, bs], raug[:],
                                         start=False, stop=True)
                        t = smp.tile([128, 256], F32, name=f"t{bt}", tag="sm",
                                     bufs=8)
                        nc.scalar.activation(out=t, in_=dp[:], func=AF.Relu)
                        nc.scalar.activation(out=t, in_=t, func=AF.Sqrt)
                        parts.append(t)
                    dtot = smp.tile([128, 256], F32, name=f"dt{bt}", tag="dt",
                                    bufs=4)
                    nc.vector.tensor_add(dtot, parts[0], parts[1])

                    # softmax chain (ACT/DVE only)
                    mn = stats.tile([128, 1], F32)
                    nc.vector.tensor_reduce(out=mn, in_=dtot,
                                            axis=mybir.AxisListType.X,
                                            op=ALU.min)
                    mb = stats.tile([128, 1], F32)
                    nc.vector.tensor_mul(mb, mn, invts)
                    e = smp.tile([128, 256], F32, name=f"e{bt}", tag="e",
                                 bufs=2)
                    s0 = stats.tile([128, 1], F32)
                    nc.scalar.activation(out=e, in_=dtot, func=AF.Exp,
                                         bias=mb[:, 0:1], scale=ninvts[:, 0:1],
                                         accum_out=s0)
                    eg = smp.tile([128, 256], F32, name=f"eg{bt}", tag="eg",
                                  bufs=2)
                    nc.vector.tensor_mul(eg, e, gatebs)
                    s1 = stats.tile([128, 1], F32)
                    nc.vector.tensor_reduce(out=s1, in_=eg,
                                            axis=mybir.AxisListType.X,
                                            op=ALU.add)
                    t3 = stats.tile([128, 1], F32)
                    nc.vector.tensor_scalar(out=t3, in0=s0, scalar1=1e-8,
                                            scalar2=None, op0=ALU.mult)
                    den = stats.tile([128, 1], F32)
                    nc.vector.tensor_add(den, s1, t3)
                    wi = stats.tile([128, 1], F32)
                    nc.vector.reciprocal(wi, den)
                    wn = smp.tile([128, 256], F32, name=f"wn{bt}", tag="wn",
                                  bufs=4)
                    nc.vector.tensor_scalar(out=wn, in0=eg, scalar1=wi[:, 0:1],
                                            scalar2=None, op0=ALU.mult)
                    wns[p] = wn

            tail_b(BL // 128 - 1)

    nc.finalize()
    return nc


# --------------------------------------------------------------------------
# entry point
# --------------------------------------------------------------------------

def kernel(**inputs):
    xim, shared = _prep_host(inputs)
    if 'nc' not in _CACHE:
        _CACHE['nc'] = _build_nc()
    nc = _CACHE['nc']
    in_maps = []
    for c in range(NCORES):
        m = dict(shared)
        m['xim'] = np.ascontiguousarray(xim[c * BL:(c + 1) * BL])
        in_maps.append(m)
    res = run_bass_kernel_spmd(nc, in_maps, list(range(NCORES)))
    return np.concatenate([res.results[c]['out'] for c in range(NCORES)], 0)



# revision 35
# speedup vs baseline: 1.0502x; 1.0502x over previous
"""Trainium2 Bass kernel for nn_CLEAR_45561013076524 (vq_codebook).

Pure data-parallel over 8 NeuronCores: each core computes 512 images of the
conv-encoder -> SoftSOM -> (collapsed) classifier pipeline.

Mathematical simplifications (validated numerically against the reference):
  * The node-attention block has n_nodes=1, so its softmax is identically 1
    and `fused == blended` tiled 4x.  Therefore
       logits = blended @ sum_h clf_w[h*256:(h+1)*256] + clf_b
    and y/class_emb/query_*/attn_*/node_emb are dead inputs.
  * conv1 (5x5, stride 2, pad 1) is one K=75 im2col matmul (host-built
    patches), zero-padded to K=128 so the PE stays at its full 2.4 GHz
    p-state (K<128 throttles the array to 1.2 GHz, measured).
  * cdist^2 is computed as one accumulated matmul chain by augmenting the
    contraction with ones/|z|^2 rows against |c|^2/ones columns and
    pre-scaling c^T by -2.

Matmul convention: out[M,N] = lhsT[K,M].T @ rhs[K,N], K on SBUF partitions.

Perf structure (all measured on HW via microbenchmarks):
  * The PE throttles to 1.2 GHz whenever fewer than ~128 array rows are
    active, so every matmul in the hot path is arranged to keep all 128
    rows busy (zero-padding K and zero-masked weight rows where needed).
  * conv1 lhsT is [128,128] = 4 column-stacked copies of the weights, so the
    single matmul also materializes 4 replicas of h1 across the 4 partition
    quadrants -- free input replication for conv2's row-tiling.
  * conv2/conv3: the 9 taps run on 2 concurrent 64-row PE tiles via
    tile_position=(64i,0) (conv2's K=32 zero-padded to 64), accumulating
    into 2 PSUM banks that are merged at eviction (ACT copy + DVE add +
    DVE fused bias-relu, chosen to balance the two PSUM-capable engines).
  * conv4 is K=128/M=128/N=512 at full-clock cadence (~248ns incl. the
    3-dim access-pattern walker overhead).
  * enc runs "swapped": h4 column tiles are the stationary operand and
    enc_w streams as the moving operand with N=256, so the per-matmul
    weight load hides under the matmul; the phase is enc_w-DMA-bound.
  * The SoftSOM tail is split into distance / softmax / transpose passes so
    the in-order PE never stalls behind the ACT/DVE softmax chain.
"""

import numpy as np
import ml_dtypes

import concourse.bass as bass
from concourse import bacc
from concourse import mybir
from concourse.tile import TileContext
from concourse.bass_utils import run_bass_kernel_spmd
from concourse.masks import make_identity

BF16NP = ml_dtypes.bfloat16
F8NP = ml_dtypes.float8_e4m3
DRPM = mybir.MatmulPerfMode.DoubleRow
W3 = 12                   # padded x width of the h3 [c, y, x, img] layout
F32 = mybir.dt.float32
F32R = mybir.dt.float32r
BF = mybir.dt.bfloat16
F8 = mybir.dt.float8e4
AF = mybir.ActivationFunctionType
ALU = mybir.AluOpType

NCORES = 8
B = 4096
BL = B // NCORES          # images per core
CH = 64                   # chunk (images) for conv3/conv4
SUB = 32                  # sub-chunk for conv1/conv2
PAIR = 2 * CH             # images per enc pass

OFF9 = [(ky, kx) for ky in range(3) for kx in range(3)]

# DR tap-coverage tables: group shifts (sigma) and per-matmul (base, delta)
W1X = 18                  # x pitch of h1 [c, 17y, 18x, img]
SIG2 = [(0, 0), (0, 1), (1, 0), (1, 1)]
M2 = [((0, 0), (2, 0)), ((0, 2), (2, 0))]
SIG3 = [(0, 0), (1, 0)]
M3 = [((0, 0), (0, 1)), ((0, 2), (2, -2)), ((2, 1), (0, 1))]

_CACHE = {}


# --------------------------------------------------------------------------
# host-side input preparation (layout only / tiny parameter math)
# --------------------------------------------------------------------------

def _prep_host(inputs):
    f32 = np.float32
    x = np.ascontiguousarray(np.asarray(inputs['x'], f32))
    xp = np.zeros((B, 3, 34, 34), f32)
    xp[:, :, 1:33, 1:33] = x
    # conv1 im2col on host (pure gather): xim[b, (ci,ky,kx), (oy,ox)]
    from numpy.lib.stride_tricks import sliding_window_view
    win = sliding_window_view(xp, (5, 5), axis=(2, 3))[:, :, ::2, ::2]
    xim = np.zeros((B, 128, 225), F8NP)     # K pre-padded to 128 rows
    xim[:, :75] = win.transpose(0, 1, 4, 5, 2, 3).reshape(
        B, 75, 225).astype(F8NP)

    c1w = np.asarray(inputs['conv1_w'], f32)
    w1 = c1w.transpose(1, 2, 3, 0).reshape(75, 32)
    w1p = np.zeros((128, 128), f32)           # K padded to 128, M tiled 4x
    for g in range(4):
        w1p[:75, 32 * g:32 * g + 32] = w1
    w1p = w1p.astype(F8NP)
    b1r = np.tile(np.asarray(inputs['conv1_b'], f32), 4).reshape(128, 1)

    # conv2/conv3 as fp8 DoubleRow matmuls over the [c, y, x, img] layout.
    # Partition group g of the input holds a copy of the activation shifted
    # by sigma_g; a DR matmul at base tap (by,bx) with slot delta (dy,dx)
    # covers taps (by+dy*s+sy_g, bx+dx*s+sx_g) -- weights of out-of-range
    # taps are zeroed.  Coverage of all 9 taps is asserted below.
    def pack_dr(w, SIG, MM, gsz):
        nmm = len(MM)
        out = np.zeros((nmm, 128, 2, w.shape[0]), f32)
        used = []
        for m, ((by, bx), (dy, dx)) in enumerate(MM):
            for s in range(2):
                for g, (sy, sx) in enumerate(SIG):
                    ky, kx = by + dy * s + sy, bx + dx * s + sx
                    if 0 <= ky < 3 and 0 <= kx < 3:
                        out[m, gsz * g:gsz * (g + 1), s] = w[:, :, ky, kx].T
                        used.append((ky, kx))
        assert sorted(used) == [(a, b) for a in range(3) for b in range(3)]
        # partition dim first: [128, nmm, 2, M]
        return np.ascontiguousarray(out.transpose(1, 0, 2, 3)).astype(F8NP)

    # conv2 weights M-stacked 2x: PSUM rows 64-127 are a copy that gets
    # evicted straight into h2's shifted-replica partition group.
    w2n1 = pack_dr(np.asarray(inputs['conv2_w'], f32), SIG2, M2, 32)
    w2n = np.concatenate([w2n1, w2n1], axis=3)             # [128, 2, 2, 128]
    b2r = np.tile(np.asarray(inputs['conv2_b'], f32), 2).reshape(128, 1)
    w3n = pack_dr(np.asarray(inputs['conv3_w'], f32), SIG3, M3, 64)
    b3r = np.asarray(inputs['conv3_b'], f32).reshape(128, 1)

    w4f = np.asarray(inputs['conv4_w'], f32)                # [256,128,3,3]
    w4 = np.ascontiguousarray(
        w4f.reshape(2, 128, 128, 3, 3).transpose(2, 3, 4, 0, 1)
        .reshape(128, 9, 2, 128)).astype(F8NP)
    b4 = np.ascontiguousarray(
        np.asarray(inputs['conv4_b'], f32).reshape(2, 128).T)  # [128,2]

    # enc (swapped): moving operand encw_m[ct, y, c, x, dout]
    ew = np.asarray(inputs['enc_w'], f32).reshape(2, 128, 8, 8, 256)
    encw = np.ascontiguousarray(ew.transpose(0, 2, 1, 3, 4)).astype(F8NP)
    encb = np.broadcast_to(np.asarray(inputs['enc_b'], f32), (128, 256)).copy()

    nf = np.asarray(inputs['node_fc_w'], f32).reshape(2, 128, 2, 128)
    nfc = np.ascontiguousarray(nf.transpose(1, 0, 2, 3))       # [k,kt,mt,m]
    nfcb = np.ascontiguousarray(
        np.asarray(inputs['node_fc_b'], f32).reshape(2, 128).T)

    protos = np.asarray(inputs['protos'], f32)
    grid = np.asarray(inputs['grid_pos'], f32)

    def dist_rhs(c):
        # rp[k, kt, n] = -2*c[n, kt*128+k]
        # aug (K padded to 128 to keep the PE p-state up):
        #   row0 = |c|^2 (pairs with the all-ones lhsT row)
        #   row1 = ones  (pairs with the |z|^2 lhsT row)
        rp = np.ascontiguousarray(
            (-2.0 * c.T).reshape(2, 128, 256).transpose(1, 0, 2))
        aug = np.zeros((128, 256), f32)
        aug[0] = (c * c).sum(1)
        aug[1] = 1.0
        return rp.astype(f32), aug.astype(f32)

    rp, rpa = dist_rhs(protos)
    rg, rga = dist_rhs(grid)

    # blended is only consumed by the classifier, so fold protos into it:
    # logits = w_norm @ (protos @ clf_sum) + clf_b
    clf_sum = np.asarray(inputs['clf_w'], f32).reshape(4, 256, 10).sum(0)
    pc = (protos.astype(np.float64) @ clf_sum.astype(np.float64)).astype(f32)
    clfs = np.ascontiguousarray(
        pc.reshape(2, 128, 10).transpose(1, 0, 2))             # [128, 2, 10]
    clfb = np.broadcast_to(np.asarray(inputs['clf_b'], f32), (128, 10)).copy()

    gate = 1.0 / (1.0 + np.exp(-np.asarray(inputs['gate_logits'], np.float64)))
    gateb = np.broadcast_to(gate.astype(f32), (128, 256)).copy()

    traw = float(np.asarray(inputs['temp_raw']).reshape(-1)[0])
    temp = 1.0 / (1.0 + np.exp(-traw)) * (1.0 - 0.001) + 0.001
    invt = np.full((128, 1), 1.0 / temp, f32)
    ninvt = np.full((128, 1), -1.0 / temp, f32)

    shared = dict(w1=w1p, w2=w2n, w3=w3n, w4=w4,
                  onesr=np.ones((1, 512), f32), onescol=np.ones((128, 1), f32),
                  b1=b1r, b2=b2r, b3=b3r,
                  b4=b4, encw=encw, encb=encb, nfc=nfc, nfcb=nfcb,
                  rp=rp, rpa=rpa, rg=rg, rga=rga,
                  clfs=clfs, clfb=clfb, gateb=gateb, invt=invt, ninvt=ninvt)
    return xim, shared


# --------------------------------------------------------------------------
# device program
# --------------------------------------------------------------------------

def _build_nc():
    nc = bacc.Bacc(None, target_bir_lowering=False)
    P = nc.declare_dram_parameter
    xim = P("xim", [BL, 128, 225], F8, isOutput=False)
    w1 = P("w1", [128, 128], F8, isOutput=False)
    w2 = P("w2", [128, 2, 2, 128], F8, isOutput=False)
    w3 = P("w3", [128, 3, 2, 128], F8, isOutput=False)
    w4 = P("w4", [128, 9, 2, 128], F8, isOutput=False)
    b1 = P("b1", [128, 1], F32, isOutput=False)
    b2 = P("b2", [128, 1], F32, isOutput=False)
    b3 = P("b3", [128, 1], F32, isOutput=False)
    b4 = P("b4", [128, 2], F32, isOutput=False)
    encw = P("encw", [2, 8, 128, 8, 256], F8, isOutput=False)
    encb = P("encb", [128, 256], F32, isOutput=False)
    nfc = P("nfc", [128, 2, 2, 128], F32, isOutput=False)
    nfcb = P("nfcb", [128, 2], F32, isOutput=False)
    rp = P("rp", [128, 2, 256], F32, isOutput=False)
    rpa = P("rpa", [128, 256], F32, isOutput=False)
    rg = P("rg", [128, 2, 256], F32, isOutput=False)
    rga = P("rga", [128, 256], F32, isOutput=False)
    clfs = P("clfs", [128, 2, 10], F32, isOutput=False)
    clfb = P("clfb", [128, 10], F32, isOutput=False)
    gateb = P("gateb", [128, 256], F32, isOutput=False)
    invt = P("invt", [128, 1], F32, isOutput=False)
    onesr = P("onesr", [1, 512], F32, isOutput=False)
    onescol = P("onescol", [128, 1], F32, isOutput=False)
    ninvt = P("ninvt", [128, 1], F32, isOutput=False)
    outd = P("out", [BL, 10], F32, isOutput=True)

    with TileContext(nc) as tc:
        with (tc.tile_pool(name="consts", bufs=1) as consts,
              tc.tile_pool(name="acts", bufs=1) as acts,
              tc.tile_pool(name="encwp", bufs=6) as encwp,
              tc.tile_pool(name="cvtmp", bufs=6) as cvtmp,
              tc.tile_pool(name="smp", bufs=3) as smp,
              tc.tile_pool(name="stats", bufs=8) as stats,
              tc.tile_pool(name="outp", bufs=2) as outp,
              tc.tile_pool(name="psA", bufs=6, space="PSUM") as psA,
              tc.tile_pool(name="psB", bufs=2, space="PSUM") as psB):

            dma = nc.sync.dma_start

            # ---- conv1-critical loads first (everything else overlaps) ----
            w1s = consts.tile([128, 128], F8); dma(out=w1s, in_=w1[:])
            b1s = consts.tile([128, 1], F32); dma(out=b1s, in_=b1[:])
            pts = []
            for i in range(2):
                t = acts.tile([128, SUB, 15, 15], F8, name=f"pt{i}")
                pts.append(t)

            def load_patches(b0, pt):
                base = xim[b0, 0, 0]
                src = bass.AP(
                    tensor=base.tensor, offset=base.offset,
                    ap=[[225, 128], [128 * 225, SUB], [1, 225]])
                dma(out=pt[:], in_=src)

            load_patches(0, pts[0])

            # ---- remaining constants --------------------------------------
            w2s = consts.tile([128, 2, 2, 128], F8); dma(out=w2s, in_=w2[:])
            w3s = consts.tile([128, 3, 2, 128], F8); dma(out=w3s, in_=w3[:])
            w4s = consts.tile([128, 9, 2, 128], F8); dma(out=w4s, in_=w4[:])
            b2s = consts.tile([128, 1], F32); dma(out=b2s, in_=b2[:])
            b3s = consts.tile([128, 1], F32); dma(out=b3s, in_=b3[:])
            b4s = consts.tile([128, 2], F32); dma(out=b4s, in_=b4[:])
            encbs = consts.tile([128, 256], F32); dma(out=encbs, in_=encb[:])
            ident = consts.tile([128, 128], F32)
            make_identity(nc, ident)

            # ---- persistent activation tensors ----------------------------
            # h1 is [c, y, img, x] (x-innermost so conv1 evictions write 15B
            # runs); h2/h3 are [c, y, x, img].  Partition group g holds the
            # activation shifted by sigma_g, filled by flat-shift SBUF-SBUF
            # DMAs (pad columns are zero, so the img-boundary wrap of the
            # flat shift lands on values that are only read via zero
            # weights).
            h1na = []
            for i in range(2):
                t = acts.tile([128, 17, CH, W1X], F8, name=f"h1n{i}")
                # buffer 0 is needed immediately (fast DVE memset); buffer 1
                # only at chunk 1, so its memset hides on gpsimd
                (nc.vector if i == 0 else nc.gpsimd).memset(t, 0.0)
                h1na.append(t)
            h2n = acts.tile([128, 10, W3, CH], F8)
            nc.gpsimd.memset(h2n, 0.0)
            h3n = acts.tile([128, 10, W3, CH], F8)     # [c, y, x, img]
            nc.gpsimd.memset(h3n, 0.0)
            h4t = acts.tile([128, 2, 8, 8, PAIR], F8)  # [c, ct, y, x, b]
            z0b = acts.tile([128, 4, 256], F32)        # [b, pair, dout]
            z0T = acts.tile([128, 2, BL], F32)
            zT = acts.tile([128, 2, BL], F32)
            wT = acts.tile([128, 2, BL], F32)

            # softsom constants on the gpsimd queue so they overlap the conv
            # pipeline without delaying patch loads on the sync queue
            gdma = nc.gpsimd.dma_start
            nfcs = consts.tile([128, 2, 2, 128], F32); gdma(out=nfcs, in_=nfc[:])
            nfcbs = consts.tile([128, 2], F32); gdma(out=nfcbs, in_=nfcb[:])
            rps = consts.tile([128, 2, 256], F32); gdma(out=rps, in_=rp[:])
            rpas = consts.tile([128, 256], F32); gdma(out=rpas, in_=rpa[:])
            rgs = consts.tile([128, 2, 256], F32); gdma(out=rgs, in_=rg[:])
            rgas = consts.tile([128, 256], F32); gdma(out=rgas, in_=rga[:])
            clfss = consts.tile([128, 2, 10], F32); gdma(out=clfss, in_=clfs[:])
            clfbs = consts.tile([128, 10], F32); gdma(out=clfbs, in_=clfb[:])
            gatebs = consts.tile([128, 256], F32); gdma(out=gatebs, in_=gateb[:])
            invts = consts.tile([128, 1], F32); gdma(out=invts, in_=invt[:])
            ninvts = consts.tile([128, 1], F32); gdma(out=ninvts, in_=ninvt[:])
            ones_col = consts.tile([128, 1], F32)
            gdma(out=ones_col, in_=onescol[:])
            z2row = consts.tile([1, BL], F32)    # |z|^2 per image
            aug2 = consts.tile([128, BL], F32)   # K-padded aug lhsT
            nc.gpsimd.memset(aug2, 0.0)
            gdma(out=aug2[0:1], in_=onesr[:])

            rowsz = CH * W1X
            wns = {}

            def tail_b(p):
                # transposes + logits + out for a tile whose softmax chain
                # finished during the preceding chunks (PE never waits)
                bs = slice(p * 128, (p + 1) * 128)
                wn = wns.pop(p)
                for kt in range(2):
                    tp = psA.tile([128, 128], F32, tag="ps")
                    nc.tensor.transpose(
                        tp[:], wn[:, kt * 128:(kt + 1) * 128], ident[:])
                    nc.vector.tensor_copy(out=wT[:, kt, bs], in_=tp[:])
                lg = psA.tile([128, 10], F32, tag="ps")
                for kt in range(2):
                    nc.tensor.matmul(lg[:], wT[:, kt, bs], clfss[:, kt],
                                     start=(kt == 0), stop=(kt == 1))
                ot = outp.tile([128, 10], F32)
                nc.vector.tensor_add(ot, lg[:], clfbs)
                dma(out=outd[p * 128:(p + 1) * 128], in_=ot)

            def conv1_phase(c):
                # conv1: single K=128(padded) matmul per image pair, evicted
                # into partition group 0 of this chunk's h1 buffer; groups
                # 1..3 (sigma-shifted replicas) filled by flat-shift DMAs on
                # the scalar HWDGE queue, issued per s-block so the first
                # half flies while the second half's matmuls run.
                hb = h1na[c % 2]
                for s in range(CH // SUB):
                    b0 = c * CH + s * SUB
                    pt = pts[(2 * c + s) % 2]
                    if c == 0 and s == 0:
                        for q in range(4):   # split so the first MM starts asap
                            base = xim[b0 + 8 * q, 0, 0]
                            src = bass.AP(
                                tensor=base.tensor, offset=base.offset,
                                ap=[[225, 128], [128 * 225, 8], [1, 225]])
                            dma(out=pt[:, 8 * q:8 * (q + 1)], in_=src)
                    else:
                        load_patches(b0, pt)
                    for j in range(SUB // 2):
                        ps1 = psA.tile([128, 2, 15, 15], F32, tag="ps")
                        nc.tensor.matmul(ps1[:], w1s[:], pt[:, 2 * j:2 * j + 2],
                                         start=True, stop=True)
                        ib = s * SUB + 2 * j
                        dst1 = hb[0:32, 1:16, ib:ib + 2, 1:16].rearrange(
                            "p y b x -> p b y x")
                        if j % 2 == 0:
                            nc.scalar.activation(out=dst1, in_=ps1[0:32],
                                                 func=AF.Relu,
                                                 bias=b1s[0:32, 0:1])
                        else:
                            nc.vector.tensor_scalar(
                                out=dst1, in0=ps1[0:32], scalar1=b1s[0:32, 0:1],
                                scalar2=0.0, op0=ALU.add, op1=ALU.max)
                    # this s-block's slice of the shifted replicas
                    seg = SUB * W1X
                    src0 = hb[0:32, 0, 0, 0]
                    for g, (sy, sx) in enumerate(SIG2[1:], start=1):
                        rows = 17 - sy
                        cnt = seg - sx
                        dstg = hb[32 * g:32 * (g + 1), 0, 0, 0]
                        dma(
                            out=bass.AP(
                                tensor=dstg.tensor,
                                offset=dstg.offset + s * seg,
                                ap=[list(dstg.ap[0]), [rowsz, rows], [1, cnt]]),
                            in_=bass.AP(
                                tensor=src0.tensor,
                                offset=src0.offset + sy * rowsz + s * seg + sx,
                                ap=[list(src0.ap[0]), [rowsz, rows], [1, cnt]]))

            # ---- conv/enc pipeline over image chunks, conv1 one chunk
            # ahead so its replica DMAs hide under conv2..conv4 PE work ----
            conv1_phase(0)
            for c in range(BL // CH):
                pb = (c % 2) * CH
                h1c = h1na[c % 2]
                if c + 1 < BL // CH:
                    conv1_phase(c + 1)

                # conv2: stride-2 fp8 DoubleRow, 2 matmuls per output row;
                # N enumerates (img, x), the evictions transpose to (x, img).
                # M is double-stacked: PSUM rows 64-127 evict straight into
                # h2's shifted-replica partition group (one row up).
                for y in range(8):
                    p2 = psA.tile([128, CH, 8], F32, tag="ps", name="p2")
                    for m, ((by, bx), (dy, dx)) in enumerate(M2):
                        base = h1c[:, 2 * y + by, 0, bx]
                        delta = (dy * rowsz + dx)
                        rhs = bass.AP(
                            tensor=base.tensor, offset=base.offset,
                            ap=[list(base.ap[0]), [delta, 2],
                                [W1X, CH], [2, 8]])
                        nc.tensor.matmul(p2[:], w2s[:, m], rhs,
                                         start=(m == 0), stop=(m == len(M2) - 1),
                                         perf_mode=DRPM)
                    dst2 = h2n[0:64, 1 + y, 1:9, :]
                    src2 = p2[0:64].rearrange("p i x -> p x i")
                    dst2b = h2n[64:128, y, 1:9, :]
                    src2b = p2[64:128].rearrange("p i x -> p x i")
                    if y % 2 == 0:
                        nc.scalar.activation(out=dst2, in_=src2,
                                             func=AF.Relu, bias=b2s[0:64, 0:1])
                        nc.vector.tensor_scalar(
                            out=dst2b, in0=src2b, scalar1=b2s[64:128, 0:1],
                            scalar2=0.0, op0=ALU.add, op1=ALU.max)
                    else:
                        nc.vector.tensor_scalar(
                            out=dst2, in0=src2, scalar1=b2s[0:64, 0:1],
                            scalar2=0.0, op0=ALU.add, op1=ALU.max)
                        nc.scalar.activation(out=dst2b, in_=src2b,
                                             func=AF.Relu, bias=b2s[64:128, 0:1])

                # conv3: fp8 DoubleRow, 3 matmuls per output row
                for y in range(8):
                    p3 = psA.tile([128, 8, CH], F32, tag="ps", name="p3")
                    for m, ((by, bx), (dy, dx)) in enumerate(M3):
                        base = h2n[:, y + by, bx, 0]
                        delta = (dy * W3 + dx) * CH
                        rhs = bass.AP(
                            tensor=base.tensor, offset=base.offset,
                            ap=[list(base.ap[0]), [delta, 2],
                                [CH, 8], [1, CH]])
                        nc.tensor.matmul(p3[:], w3s[:, m], rhs,
                                         start=(m == 0), stop=(m == len(M3) - 1),
                                         perf_mode=DRPM)
                    dst3 = h3n[:, 1 + y, 1:9, :]
                    if y % 2 == 0:
                        nc.vector.tensor_scalar(
                            out=dst3, in0=p3[:], scalar1=b3s[:, 0:1],
                            scalar2=0.0, op0=ALU.add, op1=ALU.max)
                    else:
                        nc.scalar.activation(out=dst3, in_=p3[:],
                                             func=AF.Relu, bias=b3s[:, 0:1])

                # conv4: fp8 DoubleRow over tap pairs.  Per output row y,
                # 4 DR matmuls (flat taps (2i,2i+1), slot stride = the tap
                # offset delta in the [c,y,x,img] layout) + 1 plain fp8
                # matmul (tap 8) accumulate K=9*128 into one [128,8x,64b]
                # PSUM bank.
                for mt in range(2):
                    for y in range(8):
                        pa = psA.tile([128, 8, CH], F32, tag="ps",
                                      name="pc4")
                        for i in range(4):
                            ky0, kx0 = divmod(2 * i, 3)
                            ky1, kx1 = divmod(2 * i + 1, 3)
                            base = h3n[:, y + ky0, kx0, 0]
                            delta = ((ky1 - ky0) * W3 + (kx1 - kx0)) * CH
                            rhs = bass.AP(
                                tensor=base.tensor, offset=base.offset,
                                ap=[list(base.ap[0]), [delta, 2],
                                    [CH, 8], [1, CH]])
                            nc.tensor.matmul(pa[:],
                                             w4s[:, 2 * i:2 * i + 2, mt],
                                             rhs, start=(i == 0), stop=False,
                                             perf_mode=DRPM)
                        nc.tensor.matmul(pa[:], w4s[:, 8, mt],
                                         h3n[:, y + 2, 2:10, :],
                                         start=False, stop=True)
                        dst = h4t[:, mt, y, :, pb:pb + CH]
                        if y % 2 == 0:
                            nc.scalar.activation(
                                out=dst, in_=pa[:],
                                func=AF.Relu, bias=b4s[:, mt:mt + 1])
                        else:
                            nc.vector.tensor_scalar(
                                out=dst, in0=pa[:],
                                scalar1=b4s[:, mt:mt + 1], scalar2=0.0,
                                op0=ALU.add, op1=ALU.max)

                # enc (swapped): h4 column-tiles stationary, enc_w moving
                if c % 2 == 1:
                    p = c // 2
                    if p >= 1:
                        tail_b(p - 1)
                    zp = psB.tile([128, 256], F32, tag="pe")
                    for ct in range(2):
                        for y in range(8):
                            ewt = encwp.tile([128, 8, 256], F8)
                            nc.gpsimd.dma_start(out=ewt, in_=encw[ct, y])
                            for xx in range(8):
                                first = (ct == 0 and y == 0 and xx == 0)
                                last = (ct == 1 and y == 7 and xx == 7)
                                nc.tensor.matmul(
                                    zp[:], h4t[:, ct, y, xx], ewt[:, xx],
                                    start=first, stop=last)
                    nc.vector.tensor_add(z0b[:, p], zp[:], encbs)
                    # transpose this pair's z0 into z0T right away so it
                    # overlaps with the next chunks' conv work
                    for kt in range(2):
                        tp = psA.tile([128, 128], F32, tag="ps")
                        nc.tensor.transpose(
                            tp[:], z0b[:, p, 128 * kt:128 * kt + 128],
                            ident[:])
                        nc.vector.tensor_copy(
                            out=z0T[:, kt, p * 128:(p + 1) * 128], in_=tp[:])

                    # ---- SoftSOM tail for this 128-image tile, inline so
                    # it overlaps the remaining chunks' conv work ----------
                    bt = p
                    bs = slice(bt * 128, (bt + 1) * 128)
                    for mt in range(2):
                        zpm = psA.tile([128, 128], F32, tag="ps",
                                       name="zpm")
                        for kt in range(2):
                            nc.tensor.matmul(zpm[:], nfcs[:, kt, mt],
                                             z0T[:, kt, bs],
                                             start=(kt == 0), stop=(kt == 1))
                        nc.vector.tensor_scalar(out=zT[:, mt, bs], in0=zpm[:],
                                                scalar1=nfcbs[:, mt:mt + 1],
                                                scalar2=None, op0=ALU.add)
                    zp2 = psA.tile([1, 128], F32, tag="ps", name="zp2")
                    for kt in range(2):
                        sqk = cvtmp.tile([128, 128], F32, tag='sqk', bufs=2)
                        nc.scalar.activation(out=sqk, in_=zT[:, kt, bs],
                                             func=AF.Square)
                        nc.tensor.matmul(zp2[:], ones_col[:], sqk,
                                         start=(kt == 0), stop=(kt == 1))
                    nc.vector.tensor_copy(out=z2row[0:1, bs], in_=zp2[:])
                    dma(out=aug2[1:2, bs], in_=z2row[0:1, bs])

                    # distances
                    parts = []
                    for rmain, raug in ((rps, rpas), (rgs, rgas)):
                        dp = psA.tile([128, 256], F32, tag="ps",
                                      name=f"dp{bt}")
                        nc.tensor.matmul(dp[:], zT[:, 0, bs], rmain[:, 0],
                                         start=True, stop=False)
                        nc.tensor.matmul(dp[:], zT[:, 1, bs], rmain[:, 1],
                                         start=False, stop=False)
                        nc.tensor.matmul(dp[:], aug2[:, bs], raug[:],
                                         start=False, stop=True)
                        t = smp.tile([128, 256], F32, name=f"t{bt}", tag="sm",
                                     bufs=8)
                        nc.scalar.activation(out=t, in_=dp[:], func=AF.Relu)
                        nc.scalar.activation(out=t, in_=t, func=AF.Sqrt)
                        parts.append(t)
                    dtot = smp.tile([128, 256], F32, name=f"dt{bt}", tag="dt",
                                    bufs=4)
                    nc.vector.tensor_add(dtot, parts[0], parts[1])

                    # softmax chain (ACT/DVE only)
                    mn = stats.tile([128, 1], F32)
                    nc.vector.tensor_reduce(out=mn, in_=dtot,
                                            axis=mybir.AxisListType.X,
                                            op=ALU.min)
                    mb = stats.tile([128, 1], F32)
                    nc.vector.tensor_mul(mb, mn, invts)
                    e = smp.tile([128, 256], F32, name=f"e{bt}", tag="e",
                                 bufs=2)
                    s0 = stats.tile([128, 1], F32)
                    nc.scalar.activation(out=e, in_=dtot, func=AF.Exp,
                                         bias=mb[:, 0:1], scale=ninvts[:, 0:1],
                                         accum_out=s0)
                    eg = smp.tile([128, 256], F32, name=f"eg{bt}", tag="eg",
                                  bufs=2)
                    nc.vector.tensor_mul(eg, e, gatebs)
                    s1 = stats.tile([128, 1], F32)
                    nc.vector.tensor_reduce(out=s1, in_=eg,
                                            axis=mybir.AxisListType.X,
                                            op=ALU.add)
                    t3 = stats.tile([128, 1], F32)
                    nc.vector.tensor_scalar(out=t3, in0=s0, scalar1=1e-8,
                                            scalar2=None, op0=ALU.mult)
                    den = stats.tile([128, 1], F32)
                    nc.vector.tensor_add(den, s1, t3)
                    wi = stats.tile([128, 1], F32)
                    nc.vector.reciprocal(wi, den)
                    wn = smp.tile([128, 256], F32, name=f"wn{bt}", tag="wn",
                                  bufs=4)
                    nc.vector.tensor_scalar(out=wn, in0=eg, scalar1=wi[:, 0:1],
                                            scalar2=None, op0=ALU.mult)
                    wns[p] = wn

            tail_b(BL // 128 - 1)

    nc.finalize()
    return nc


# --------------------------------------------------------------------------
# entry point
# --------------------------------------------------------------------------

def kernel(**inputs):
    xim, shared = _prep_host(inputs)
    if 'nc' not in _CACHE:
        _CACHE['nc'] = _build_nc()
    nc = _CACHE['nc']
    in_maps = []
    for c in range(NCORES):
        m = dict(shared)
        m['xim'] = np.ascontiguousarray(xim[c * BL:(c + 1) * BL])
        in_maps.append(m)
    res = run_bass_kernel_spmd(nc, in_maps, list(range(NCORES)))
    return np.concatenate([res.results[c]['out'] for c in range(NCORES)], 0)



# revision 42
# speedup vs baseline: 1.0522x; 1.0019x over previous
"""Trainium2 Bass kernel for nn_CLEAR_45561013076524 (vq_codebook).

Pure data-parallel over 8 NeuronCores: each core computes 512 images of the
conv-encoder -> SoftSOM -> (collapsed) classifier pipeline.

Mathematical simplifications (validated numerically against the reference):
  * The node-attention block has n_nodes=1, so its softmax is identically 1
    and `fused == blended` tiled 4x.  Therefore
       logits = blended @ sum_h clf_w[h*256:(h+1)*256] + clf_b
    and y/class_emb/query_*/attn_*/node_emb are dead inputs.
  * conv1 (5x5, stride 2, pad 1) is one K=75 im2col matmul (host-built
    patches), zero-padded to K=128 so the PE stays at its full 2.4 GHz
    p-state (K<128 throttles the array to 1.2 GHz, measured).
  * cdist^2 is computed as one accumulated matmul chain by augmenting the
    contraction with ones/|z|^2 rows against |c|^2/ones columns and
    pre-scaling c^T by -2.

Matmul convention: out[M,N] = lhsT[K,M].T @ rhs[K,N], K on SBUF partitions.

Perf structure (all measured on HW via microbenchmarks):
  * The PE throttles to 1.2 GHz whenever fewer than ~128 array rows are
    active, so every matmul in the hot path is arranged to keep all 128
    rows busy (zero-padding K and zero-masked weight rows where needed).
  * conv1 lhsT is [128,128] = 4 column-stacked copies of the weights, so the
    single matmul also materializes 4 replicas of h1 across the 4 partition
    quadrants -- free input replication for conv2's row-tiling.
  * conv2/conv3: the 9 taps run on 2 concurrent 64-row PE tiles via
    tile_position=(64i,0) (conv2's K=32 zero-padded to 64), accumulating
    into 2 PSUM banks that are merged at eviction (ACT copy + DVE add +
    DVE fused bias-relu, chosen to balance the two PSUM-capable engines).
  * conv4 is K=128/M=128/N=512 at full-clock cadence (~248ns incl. the
    3-dim access-pattern walker overhead).
  * enc runs "swapped": h4 column tiles are the stationary operand and
    enc_w streams as the moving operand with N=256, so the per-matmul
    weight load hides under the matmul; the phase is enc_w-DMA-bound.
  * The SoftSOM tail is split into distance / softmax / transpose passes so
    the in-order PE never stalls behind the ACT/DVE softmax chain.
"""

import numpy as np
import ml_dtypes

import concourse.bass as bass
from concourse import bacc
from concourse import mybir
from concourse.tile import TileContext
from concourse.bass_utils import run_bass_kernel_spmd
from concourse.masks import make_identity

BF16NP = ml_dtypes.bfloat16
F8NP = ml_dtypes.float8_e4m3
DRPM = mybir.MatmulPerfMode.DoubleRow
W3 = 12                   # padded x width of the h3 [c, y, x, img] layout
F32 = mybir.dt.float32
F32R = mybir.dt.float32r
BF = mybir.dt.bfloat16
F8 = mybir.dt.float8e4
AF = mybir.ActivationFunctionType
ALU = mybir.AluOpType

NCORES = 8
B = 4096
BL = B // NCORES          # images per core
CH = 64                   # chunk (images) for conv3/conv4
SUB = 32                  # sub-chunk for conv1/conv2
PAIR = 2 * CH             # images per enc pass

OFF9 = [(ky, kx) for ky in range(3) for kx in range(3)]

# DR tap-coverage tables: group shifts (sigma) and per-matmul (base, delta)
W1X = 18                  # x pitch of h1 [c, 17y, 18x, img]
SIG2 = [(0, 0), (0, 1), (1, 0), (1, 1)]
M2 = [((0, 0), (2, 0)), ((0, 2), (2, 0))]
SIG3 = [(0, 0), (1, 0)]
M3 = [((0, 0), (0, 1)), ((0, 2), (2, -2)), ((2, 1), (0, 1))]

_CACHE = {}


# --------------------------------------------------------------------------
# host-side input preparation (layout only / tiny parameter math)
# --------------------------------------------------------------------------

def _prep_host(inputs):
    f32 = np.float32
    x = np.ascontiguousarray(np.asarray(inputs['x'], f32))
    xp = np.zeros((B, 3, 34, 34), f32)
    xp[:, :, 1:33, 1:33] = x
    # conv1 im2col on host (pure gather): xim[b, (ci,ky,kx), (oy,ox)]
    from numpy.lib.stride_tricks import sliding_window_view
    win = sliding_window_view(xp, (5, 5), axis=(2, 3))[:, :, ::2, ::2]
    xim = np.zeros((B, 128, 225), F8NP)     # K pre-padded to 128 rows
    xim[:, :75] = win.transpose(0, 1, 4, 5, 2, 3).reshape(
        B, 75, 225).astype(F8NP)

    c1w = np.asarray(inputs['conv1_w'], f32)
    w1 = c1w.transpose(1, 2, 3, 0).reshape(75, 32)
    w1p = np.zeros((128, 128), f32)           # K padded to 128, M tiled 4x
    for g in range(4):
        w1p[:75, 32 * g:32 * g + 32] = w1
    w1p = w1p.astype(F8NP)
    b1r = np.tile(np.asarray(inputs['conv1_b'], f32), 4).reshape(128, 1)

    # conv2/conv3 as fp8 DoubleRow matmuls over the [c, y, x, img] layout.
    # Partition group g of the input holds a copy of the activation shifted
    # by sigma_g; a DR matmul at base tap (by,bx) with slot delta (dy,dx)
    # covers taps (by+dy*s+sy_g, bx+dx*s+sx_g) -- weights of out-of-range
    # taps are zeroed.  Coverage of all 9 taps is asserted below.
    def pack_dr(w, SIG, MM, gsz):
        nmm = len(MM)
        out = np.zeros((nmm, 128, 2, w.shape[0]), f32)
        used = []
        for m, ((by, bx), (dy, dx)) in enumerate(MM):
            for s in range(2):
                for g, (sy, sx) in enumerate(SIG):
                    ky, kx = by + dy * s + sy, bx + dx * s + sx
                    if 0 <= ky < 3 and 0 <= kx < 3:
                        out[m, gsz * g:gsz * (g + 1), s] = w[:, :, ky, kx].T
                        used.append((ky, kx))
        assert sorted(used) == [(a, b) for a in range(3) for b in range(3)]
        # partition dim first: [128, nmm, 2, M]
        return np.ascontiguousarray(out.transpose(1, 0, 2, 3)).astype(F8NP)

    # conv2 weights M-stacked 2x: PSUM rows 64-127 are a copy that gets
    # evicted straight into h2's shifted-replica partition group.
    w2n1 = pack_dr(np.asarray(inputs['conv2_w'], f32), SIG2, M2, 32)
    w2n = np.concatenate([w2n1, w2n1], axis=3)             # [128, 2, 2, 128]
    b2r = np.tile(np.asarray(inputs['conv2_b'], f32), 2).reshape(128, 1)
    w3n = pack_dr(np.asarray(inputs['conv3_w'], f32), SIG3, M3, 64)
    b3r = np.asarray(inputs['conv3_b'], f32).reshape(128, 1)

    w4f = np.asarray(inputs['conv4_w'], f32)                # [256,128,3,3]
    w4 = np.ascontiguousarray(
        w4f.reshape(2, 128, 128, 3, 3).transpose(2, 3, 4, 0, 1)
        .reshape(128, 9, 2, 128)).astype(F8NP)
    b4 = np.ascontiguousarray(
        np.asarray(inputs['conv4_b'], f32).reshape(2, 128).T)  # [128,2]

    # enc (swapped): moving operand encw_m[ct, y, c, x, dout]
    ew = np.asarray(inputs['enc_w'], f32).reshape(2, 128, 8, 8, 256)
    encw = np.ascontiguousarray(ew.transpose(0, 2, 1, 3, 4)).astype(F8NP)
    encb = np.broadcast_to(np.asarray(inputs['enc_b'], f32), (128, 256)).copy()

    nf = np.asarray(inputs['node_fc_w'], f32).reshape(2, 128, 2, 128)
    nfc = np.ascontiguousarray(nf.transpose(1, 0, 2, 3))       # [k,kt,mt,m]
    nfcb = np.ascontiguousarray(
        np.asarray(inputs['node_fc_b'], f32).reshape(2, 128).T)

    protos = np.asarray(inputs['protos'], f32)
    grid = np.asarray(inputs['grid_pos'], f32)

    def dist_rhs(c):
        # rp[k, kt, n] = -2*c[n, kt*128+k]
        # aug (K padded to 128 to keep the PE p-state up):
        #   row0 = |c|^2 (pairs with the all-ones lhsT row)
        #   row1 = ones  (pairs with the |z|^2 lhsT row)
        rp = np.ascontiguousarray(
            (-2.0 * c.T).reshape(2, 128, 256).transpose(1, 0, 2))
        aug = np.zeros((128, 256), f32)
        aug[0] = (c * c).sum(1)
        aug[1] = 1.0
        return rp.astype(f32), aug.astype(f32)

    rp, rpa = dist_rhs(protos)
    rg, rga = dist_rhs(grid)

    # blended is only consumed by the classifier, so fold protos into it:
    # logits = w_norm @ (protos @ clf_sum) + clf_b
    clf_sum = np.asarray(inputs['clf_w'], f32).reshape(4, 256, 10).sum(0)
    pc = (protos.astype(np.float64) @ clf_sum.astype(np.float64)).astype(f32)
    clfs = np.ascontiguousarray(
        pc.reshape(2, 128, 10).transpose(1, 0, 2))             # [128, 2, 10]
    clfb = np.broadcast_to(np.asarray(inputs['clf_b'], f32), (128, 10)).copy()

    gate = 1.0 / (1.0 + np.exp(-np.asarray(inputs['gate_logits'], np.float64)))
    gateb = np.broadcast_to(gate.astype(f32), (128, 256)).copy()

    traw = float(np.asarray(inputs['temp_raw']).reshape(-1)[0])
    temp = 1.0 / (1.0 + np.exp(-traw)) * (1.0 - 0.001) + 0.001
    invt = np.full((128, 1), 1.0 / temp, f32)
    ninvt = np.full((128, 1), -1.0 / temp, f32)

    shared = dict(w1=w1p, w2=w2n, w3=w3n, w4=w4,
                  onesr=np.ones((1, 512), f32), onescol=np.ones((128, 1), f32),
                  b1=b1r, b2=b2r, b3=b3r,
                  b4=b4, encw=encw, encb=encb, nfc=nfc, nfcb=nfcb,
                  rp=rp, rpa=rpa, rg=rg, rga=rga,
                  clfs=clfs, clfb=clfb, gateb=gateb, invt=invt, ninvt=ninvt)
    return xim, shared


# --------------------------------------------------------------------------
# device program
# --------------------------------------------------------------------------

def _build_nc(uniform_gate=True):
    nc = bacc.Bacc(None, target_bir_lowering=False)
    P = nc.declare_dram_parameter
    xim = P("xim", [BL, 128, 225], F8, isOutput=False)
    w1 = P("w1", [128, 128], F8, isOutput=False)
    w2 = P("w2", [128, 2, 2, 128], F8, isOutput=False)
    w3 = P("w3", [128, 3, 2, 128], F8, isOutput=False)
    w4 = P("w4", [128, 9, 2, 128], F8, isOutput=False)
    b1 = P("b1", [128, 1], F32, isOutput=False)
    b2 = P("b2", [128, 1], F32, isOutput=False)
    b3 = P("b3", [128, 1], F32, isOutput=False)
    b4 = P("b4", [128, 2], F32, isOutput=False)
    encw = P("encw", [2, 8, 128, 8, 256], F8, isOutput=False)
    encb = P("encb", [128, 256], F32, isOutput=False)
    nfc = P("nfc", [128, 2, 2, 128], F32, isOutput=False)
    nfcb = P("nfcb", [128, 2], F32, isOutput=False)
    rp = P("rp", [128, 2, 256], F32, isOutput=False)
    rpa = P("rpa", [128, 256], F32, isOutput=False)
    rg = P("rg", [128, 2, 256], F32, isOutput=False)
    rga = P("rga", [128, 256], F32, isOutput=False)
    clfs = P("clfs", [128, 2, 10], F32, isOutput=False)
    clfb = P("clfb", [128, 10], F32, isOutput=False)
    gateb = P("gateb", [128, 256], F32, isOutput=False)
    invt = P("invt", [128, 1], F32, isOutput=False)
    onesr = P("onesr", [1, 512], F32, isOutput=False)
    onescol = P("onescol", [128, 1], F32, isOutput=False)
    ninvt = P("ninvt", [128, 1], F32, isOutput=False)
    outd = P("out", [BL, 10], F32, isOutput=True)

    with TileContext(nc) as tc:
        with (tc.tile_pool(name="consts", bufs=1) as consts,
              tc.tile_pool(name="acts", bufs=1) as acts,
              tc.tile_pool(name="encwp", bufs=6) as encwp,
              tc.tile_pool(name="cvtmp", bufs=6) as cvtmp,
              tc.tile_pool(name="smp", bufs=3) as smp,
              tc.tile_pool(name="stats", bufs=8) as stats,
              tc.tile_pool(name="outp", bufs=2) as outp,
              tc.tile_pool(name="psA", bufs=6, space="PSUM") as psA,
              tc.tile_pool(name="psB", bufs=2, space="PSUM") as psB):

            dma = nc.sync.dma_start

            # ---- conv1-critical loads first (everything else overlaps) ----
            w1s = consts.tile([128, 128], F8); dma(out=w1s, in_=w1[:])
            b1s = consts.tile([128, 1], F32); dma(out=b1s, in_=b1[:])
            pts = []
            for i in range(2):
                t = acts.tile([128, SUB, 15, 15], F8, name=f"pt{i}")
                pts.append(t)

            def load_patches(b0, pt, q=None):
                base = xim[b0, 0, 0]
                src = bass.AP(
                    tensor=base.tensor, offset=base.offset,
                    ap=[[225, 128], [128 * 225, SUB], [1, 225]])
                (q or dma)(out=pt[:], in_=src)

            # ---- remaining constants --------------------------------------
            w2s = consts.tile([128, 2, 2, 128], F8); dma(out=w2s, in_=w2[:])
            w3s = consts.tile([128, 3, 2, 128], F8); dma(out=w3s, in_=w3[:])
            w4s = consts.tile([128, 9, 2, 128], F8); dma(out=w4s, in_=w4[:])
            b2s = consts.tile([128, 1], F32); dma(out=b2s, in_=b2[:])
            b3s = consts.tile([128, 1], F32); dma(out=b3s, in_=b3[:])
            b4s = consts.tile([128, 2], F32); dma(out=b4s, in_=b4[:])
            encbs = consts.tile([128, 256], F32); dma(out=encbs, in_=encb[:])
            ident = consts.tile([128, 128], F32)
            make_identity(nc, ident)

            # ---- persistent activation tensors ----------------------------
            # h1 is [c, y, img, x] (x-innermost so conv1 evictions write 15B
            # runs); h2/h3 are [c, y, x, img].  Partition group g holds the
            # activation shifted by sigma_g, filled by flat-shift SBUF-SBUF
            # DMAs (pad columns are zero, so the img-boundary wrap of the
            # flat shift lands on values that are only read via zero
            # weights).
            h1na = []
            for i in range(2):
                t = acts.tile([128, 17, CH, W1X], F8, name=f"h1n{i}")
                # buffer 0 is needed immediately (fast DVE memset); buffer 1
                # only at chunk 1, so its memset hides on gpsimd
                (nc.vector if i == 0 else nc.gpsimd).memset(t, 0.0)
                h1na.append(t)
            h2n = acts.tile([128, 10, W3, CH], F8)
            nc.gpsimd.memset(h2n, 0.0)
            h3n = acts.tile([128, 10, W3, CH], F8)     # [c, y, x, img]
            nc.gpsimd.memset(h3n, 0.0)
            h4t = acts.tile([128, 2, 8, 8, PAIR], F8)  # [c, ct, y, x, b]
            z0b = acts.tile([128, 4, 256], F32)        # [b, pair, dout]
            z0T = acts.tile([128, 2, BL], F32)
            zT = acts.tile([128, 2, BL], F32)
            wT = acts.tile([128, 2, BL], F32)

            # softsom constants on the gpsimd queue so they overlap the conv
            # pipeline without delaying patch loads on the sync queue
            gdma = nc.gpsimd.dma_start
            nfcs = consts.tile([128, 2, 2, 128], F32); gdma(out=nfcs, in_=nfc[:])
            nfcbs = consts.tile([128, 2], F32); gdma(out=nfcbs, in_=nfcb[:])
            rps = consts.tile([128, 2, 256], F32); gdma(out=rps, in_=rp[:])
            rpas = consts.tile([128, 256], F32); gdma(out=rpas, in_=rpa[:])
            rgs = consts.tile([128, 2, 256], F32); gdma(out=rgs, in_=rg[:])
            rgas = consts.tile([128, 256], F32); gdma(out=rgas, in_=rga[:])
            clfss = consts.tile([128, 2, 10], F32); gdma(out=clfss, in_=clfs[:])
            clfbs = consts.tile([128, 10], F32); gdma(out=clfbs, in_=clfb[:])
            gatebs = consts.tile([128, 256], F32); gdma(out=gatebs, in_=gateb[:])
            invts = consts.tile([128, 1], F32); gdma(out=invts, in_=invt[:])
            ninvts = consts.tile([128, 1], F32); gdma(out=ninvts, in_=ninvt[:])
            ones_col = consts.tile([128, 1], F32)
            gdma(out=ones_col, in_=onescol[:])
            z2row = consts.tile([1, BL], F32)    # |z|^2 per image
            aug2 = consts.tile([128, BL], F32)   # K-padded aug lhsT
            nc.gpsimd.memset(aug2, 0.0)
            gdma(out=aug2[0:1], in_=onesr[:])

            rowsz = CH * W1X
            wns = {}

            def tail_b(p):
                # transposes + logits + out for a tile whose softmax chain
                # finished during the preceding chunks (PE never waits)
                bs = slice(p * 128, (p + 1) * 128)
                wn = wns.pop(p)
                for kt in range(2):
                    tp = psA.tile([128, 128], F32, tag="ps")
                    nc.tensor.transpose(
                        tp[:], wn[:, kt * 128:(kt + 1) * 128], ident[:])
                    nc.vector.tensor_copy(out=wT[:, kt, bs], in_=tp[:])
                lg = psA.tile([128, 10], F32, tag="ps")
                for kt in range(2):
                    nc.tensor.matmul(lg[:], wT[:, kt, bs], clfss[:, kt],
                                     start=(kt == 0), stop=(kt == 1))
                ot = outp.tile([128, 10], F32)
                nc.vector.tensor_add(ot, lg[:], clfbs)
                dma(out=outd[p * 128:(p + 1) * 128], in_=ot)

            def conv1_phase(c):
                # conv1: single K=128(padded) matmul per image pair, evicted
                # into partition group 0 of this chunk's h1 buffer; groups
                # 1..3 (sigma-shifted replicas) filled by flat-shift DMAs,
                # issued per s-block so the first half flies while the
                # second half's matmuls run.  Chunk 1 uses the scalar HWDGE
                # queue so its DMAs run parallel to chunk 0's at startup.
                qdma = nc.scalar.dma_start if c == 1 else dma
                hb = h1na[c % 2]
                for s in range(CH // SUB):
                    b0 = c * CH + s * SUB
                    pt = pts[(2 * c + s) % 2]
                    if c == 0:
                        for q in range(4):   # split so the first MM starts asap
                            base = xim[b0 + 8 * q, 0, 0]
                            src = bass.AP(
                                tensor=base.tensor, offset=base.offset,
                                ap=[[225, 128], [128 * 225, 8], [1, 225]])
                            qdma(out=pt[:, 8 * q:8 * (q + 1)], in_=src)
                    else:
                        load_patches(b0, pt, qdma)
                    for j in range(SUB // 2):
                        ps1 = psA.tile([128, 2, 15, 15], F32, tag="ps")
                        nc.tensor.matmul(ps1[:], w1s[:], pt[:, 2 * j:2 * j + 2],
                                         start=True, stop=True)
                        ib = s * SUB + 2 * j
                        dst1 = hb[0:32, 1:16, ib:ib + 2, 1:16].rearrange(
                            "p y b x -> p b y x")
                        if j % 2 == 0:
                            nc.scalar.activation(out=dst1, in_=ps1[0:32],
                                                 func=AF.Relu,
                                                 bias=b1s[0:32, 0:1])
                        else:
                            nc.vector.tensor_scalar(
                                out=dst1, in0=ps1[0:32], scalar1=b1s[0:32, 0:1],
                                scalar2=0.0, op0=ALU.add, op1=ALU.max)
                    # this s-block's slice of the shifted replicas
                    seg = SUB * W1X
                    src0 = hb[0:32, 0, 0, 0]
                    for g, (sy, sx) in enumerate(SIG2[1:], start=1):
                        rows = 17 - sy
                        cnt = seg - sx
                        dstg = hb[32 * g:32 * (g + 1), 0, 0, 0]
                        qdma(
                            out=bass.AP(
                                tensor=dstg.tensor,
                                offset=dstg.offset + s * seg,
                                ap=[list(dstg.ap[0]), [rowsz, rows], [1, cnt]]),
                            in_=bass.AP(
                                tensor=src0.tensor,
                                offset=src0.offset + sy * rowsz + s * seg + sx,
                                ap=[list(src0.ap[0]), [rowsz, rows], [1, cnt]]))

            # ---- conv/enc pipeline over image chunks, conv1 one chunk
            # ahead so its replica DMAs hide under conv2..conv4 PE work ----
            conv1_phase(0)
            for c in range(BL // CH):
                pb = (c % 2) * CH
                h1c = h1na[c % 2]
                if c + 1 < BL // CH:
                    conv1_phase(c + 1)

                # conv2: stride-2 fp8 DoubleRow, 2 matmuls per output row;
                # N enumerates (img, x), the evictions transpose to (x, img).
                # M is double-stacked: PSUM rows 64-127 evict straight into
                # h2's shifted-replica partition group (one row up).
                for y in range(8):
                    p2 = psA.tile([128, CH, 8], F32, tag="ps", name="p2")
                    for m, ((by, bx), (dy, dx)) in enumerate(M2):
                        base = h1c[:, 2 * y + by, 0, bx]
                        delta = (dy * rowsz + dx)
                        rhs = bass.AP(
                            tensor=base.tensor, offset=base.offset,
                            ap=[list(base.ap[0]), [delta, 2],
                                [W1X, CH], [2, 8]])
                        nc.tensor.matmul(p2[:], w2s[:, m], rhs,
                                         start=(m == 0), stop=(m == len(M2) - 1),
                                         perf_mode=DRPM)
                    dst2 = h2n[0:64, 1 + y, 1:9, :]
                    src2 = p2[0:64].rearrange("p i x -> p x i")
                    dst2b = h2n[64:128, y, 1:9, :]
                    src2b = p2[64:128].rearrange("p i x -> p x i")
                    if y % 2 == 0:
                        nc.scalar.activation(out=dst2, in_=src2,
                                             func=AF.Relu, bias=b2s[0:64, 0:1])
                        nc.vector.tensor_scalar(
                            out=dst2b, in0=src2b, scalar1=b2s[64:128, 0:1],
                            scalar2=0.0, op0=ALU.add, op1=ALU.max)
                    else:
                        nc.vector.tensor_scalar(
                            out=dst2, in0=src2, scalar1=b2s[0:64, 0:1],
                            scalar2=0.0, op0=ALU.add, op1=ALU.max)
                        nc.scalar.activation(out=dst2b, in_=src2b,
                                             func=AF.Relu, bias=b2s[64:128, 0:1])

                # conv3: fp8 DoubleRow, 3 matmuls per output row
                for y in range(8):
                    p3 = psA.tile([128, 8, CH], F32, tag="ps", name="p3")
                    for m, ((by, bx), (dy, dx)) in enumerate(M3):
                        base = h2n[:, y + by, bx, 0]
                        delta = (dy * W3 + dx) * CH
                        rhs = bass.AP(
                            tensor=base.tensor, offset=base.offset,
                            ap=[list(base.ap[0]), [delta, 2],
                                [CH, 8], [1, CH]])
                        nc.tensor.matmul(p3[:], w3s[:, m], rhs,
                                         start=(m == 0), stop=(m == len(M3) - 1),
                                         perf_mode=DRPM)
                    dst3 = h3n[:, 1 + y, 1:9, :]
                    if y % 2 == 0:
                        nc.vector.tensor_scalar(
                            out=dst3, in0=p3[:], scalar1=b3s[:, 0:1],
                            scalar2=0.0, op0=ALU.add, op1=ALU.max)
                    else:
                        nc.scalar.activation(out=dst3, in_=p3[:],
                                             func=AF.Relu, bias=b3s[:, 0:1])

                # conv4: fp8 DoubleRow over tap pairs.  Per output row y,
                # 4 DR matmuls (flat taps (2i,2i+1), slot stride = the tap
                # offset delta in the [c,y,x,img] layout) + 1 plain fp8
                # matmul (tap 8) accumulate K=9*128 into one [128,8x,64b]
                # PSUM bank.
                for mt in range(2):
                    for y in range(8):
                        pa = psA.tile([128, 8, CH], F32, tag="ps",
                                      name="pc4")
                        for i in range(4):
                            ky0, kx0 = divmod(2 * i, 3)
                            ky1, kx1 = divmod(2 * i + 1, 3)
                            base = h3n[:, y + ky0, kx0, 0]
                            delta = ((ky1 - ky0) * W3 + (kx1 - kx0)) * CH
                            rhs = bass.AP(
                                tensor=base.tensor, offset=base.offset,
                                ap=[list(base.ap[0]), [delta, 2],
                                    [CH, 8], [1, CH]])
                            nc.tensor.matmul(pa[:],
                                             w4s[:, 2 * i:2 * i + 2, mt],
                                             rhs, start=(i == 0), stop=False,
                                             perf_mode=DRPM)
                        nc.tensor.matmul(pa[:], w4s[:, 8, mt],
                                         h3n[:, y + 2, 2:10, :],
                                         start=False, stop=True)
                        dst = h4t[:, mt, y, :, pb:pb + CH]
                        if y % 2 == 0:
                            nc.scalar.activation(
                                out=dst, in_=pa[:],
                                func=AF.Relu, bias=b4s[:, mt:mt + 1])
                        else:
                            nc.vector.tensor_scalar(
                                out=dst, in0=pa[:],
                                scalar1=b4s[:, mt:mt + 1], scalar2=0.0,
                                op0=ALU.add, op1=ALU.max)

                # enc (swapped): h4 column-tiles stationary, enc_w moving
                if c % 2 == 1:
                    p = c // 2
                    if p >= 1:
                        tail_b(p - 1)
                    zp = psB.tile([128, 256], F32, tag="pe")
                    for ct in range(2):
                        for y in range(8):
                            ewt = encwp.tile([128, 8, 256], F8)
                            nc.gpsimd.dma_start(out=ewt, in_=encw[ct, y])
                            for xx in range(8):
                                first = (ct == 0 and y == 0 and xx == 0)
                                last = (ct == 1 and y == 7 and xx == 7)
                                nc.tensor.matmul(
                                    zp[:], h4t[:, ct, y, xx], ewt[:, xx],
                                    start=first, stop=last)
                    nc.vector.tensor_add(z0b[:, p], zp[:], encbs)
                    # transpose this pair's z0 into z0T right away so it
                    # overlaps with the next chunks' conv work
                    for kt in range(2):
                        tp = psA.tile([128, 128], F32, tag="ps")
                        nc.tensor.transpose(
                            tp[:], z0b[:, p, 128 * kt:128 * kt + 128],
                            ident[:])
                        nc.vector.tensor_copy(
                            out=z0T[:, kt, p * 128:(p + 1) * 128], in_=tp[:])

                    # ---- SoftSOM tail for this 128-image tile, inline so
                    # it overlaps the remaining chunks' conv work ----------
                    bt = p
                    bs = slice(bt * 128, (bt + 1) * 128)
                    for mt in range(2):
                        zpm = psA.tile([128, 128], F32, tag="ps",
                                       name="zpm")
                        for kt in range(2):
                            nc.tensor.matmul(zpm[:], nfcs[:, kt, mt],
                                             z0T[:, kt, bs],
                                             start=(kt == 0), stop=(kt == 1))
                        nc.vector.tensor_scalar(out=zT[:, mt, bs], in0=zpm[:],
                                                scalar1=nfcbs[:, mt:mt + 1],
                                                scalar2=None, op0=ALU.add)
                    zp2 = psA.tile([1, 128], F32, tag="ps", name="zp2")
                    for kt in range(2):
                        sqk = cvtmp.tile([128, 128], F32, tag='sqk', bufs=2)
                        nc.scalar.activation(out=sqk, in_=zT[:, kt, bs],
                                             func=AF.Square)
                        nc.tensor.matmul(zp2[:], ones_col[:], sqk,
                                         start=(kt == 0), stop=(kt == 1))
                    nc.vector.tensor_copy(out=z2row[0:1, bs], in_=zp2[:])
                    dma(out=aug2[1:2, bs], in_=z2row[0:1, bs])

                    # distances
                    parts = []
                    for rmain, raug in ((rps, rpas), (rgs, rgas)):
                        dp = psA.tile([128, 256], F32, tag="ps",
                                      name=f"dp{bt}")
                        nc.tensor.matmul(dp[:], zT[:, 0, bs], rmain[:, 0],
                                         start=True, stop=False)
                        nc.tensor.matmul(dp[:], zT[:, 1, bs], rmain[:, 1],
                                         start=False, stop=False)
                        nc.tensor.matmul(dp[:], aug2[:, bs], raug[:],
                                         start=False, stop=True)
                        t = smp.tile([128, 256], F32, name=f"t{bt}", tag="sm",
                                     bufs=8)
                        nc.scalar.activation(out=t, in_=dp[:], func=AF.Relu)
                        nc.scalar.activation(out=t, in_=t, func=AF.Sqrt)
                        parts.append(t)
                    dtot = smp.tile([128, 256], F32, name=f"dt{bt}", tag="dt",
                                    bufs=4)
                    nc.vector.tensor_add(dtot, parts[0], parts[1])

                    # softmax chain (ACT/DVE only).  No max/min stabilizer:
                    # d_total/T is bounded (measured ~[30, 36], and >= 0 by
                    # construction), so exp(-d/T) can neither overflow nor
                    # underflow f32.  The gate is sigmoid(0)=0.5 uniformly
                    # (checked on host), so softmax*gate renormalized
                    # reduces to e/sum(e) up to a 2e-8 factor.
                    e = smp.tile([128, 256], F32, name=f"e{bt}", tag="e",
                                 bufs=2)
                    s0 = stats.tile([128, 1], F32)
                    nc.scalar.activation(out=e, in_=dtot, func=AF.Exp,
                                         scale=ninvts[:, 0:1],
                                         accum_out=s0)
                    wn = smp.tile([128, 256], F32, name=f"wn{bt}", tag="wn",
                                  bufs=4)
                    if uniform_gate:
                        wi = stats.tile([128, 1], F32)
                        nc.vector.reciprocal(wi, s0)
                        nc.vector.tensor_scalar(out=wn, in0=e,
                                                scalar1=wi[:, 0:1],
                                                scalar2=None, op0=ALU.mult)
                    else:
                        eg = smp.tile([128, 256], F32, name=f"eg{bt}",
                                      tag="eg", bufs=2)
                        nc.vector.tensor_mul(eg, e, gatebs)
                        s1 = stats.tile([128, 1], F32)
                        nc.vector.tensor_reduce(out=s1, in_=eg,
                                                axis=mybir.AxisListType.X,
                                                op=ALU.add)
                        t3 = stats.tile([128, 1], F32)
                        nc.vector.tensor_scalar(out=t3, in0=s0, scalar1=1e-8,
                                                scalar2=None, op0=ALU.mult)
                        den = stats.tile([128, 1], F32)
                        nc.vector.tensor_add(den, s1, t3)
                        wi = stats.tile([128, 1], F32)
                        nc.vector.reciprocal(wi, den)
                        nc.vector.tensor_scalar(out=wn, in0=eg,
                                                scalar1=wi[:, 0:1],
                                                scalar2=None, op0=ALU.mult)
                    wns[p] = wn

            tail_b(BL // 128 - 1)

    nc.finalize()
    return nc


# --------------------------------------------------------------------------
# entry point
# --------------------------------------------------------------------------

def kernel(**inputs):
    xim, shared = _prep_host(inputs)
    gl = np.asarray(inputs['gate_logits'], np.float64)
    g0 = 1.0 / (1.0 + np.exp(-gl.reshape(-1)[0]))
    # the folded 1e-8 renorm term is only negligible if the gate isn't tiny
    ug = bool(np.all(gl == gl.reshape(-1)[0])) and g0 > 1e-4
    key = ('nc', ug)
    if key not in _CACHE:
        _CACHE[key] = _build_nc(uniform_gate=ug)
    nc = _CACHE[key]
    in_maps = []
    for c in range(NCORES):
        m = dict(shared)
        m['xim'] = np.ascontiguousarray(xim[c * BL:(c + 1) * BL])
        in_maps.append(m)
    res = run_bass_kernel_spmd(nc, in_maps, list(range(NCORES)))
    return np.concatenate([res.results[c]['out'] for c in range(NCORES)], 0)

